# revision 1
# baseline (speedup 1.0000x reference)
"""Trainium2 Bass kernel for nn_Normalizer (annealed top-k masking normalizer).

Math notes (derived from the reference):
  - The reference loop maintains b = -relu(score+a), so score+b = min(score,-a)
    and each iteration is s_t = sum(exp(min(score,-a)/theta_t)).
  - In exp-space with F_t = exp(sm/theta_t) (sm = masked score, unclipped):
        s_t = sum(min(F_t, cv_t)),   cv_t = exp(-a_{t-1}/theta_t)
    and since a_t = theta_t*log(k/s_t'), the clip level updates with plain
    arithmetic:  cv_t = (s_{t-1}'/k)^(theta_{t-1}/theta_t)  -- no log/exp.
  - For t>=8 theta_t == 0.3 is constant, so E = exp(sm/0.3) is computed once
    and each iteration is one fused DVE min+row-sum; the exponent ratio is 1
    so cv_t = s'/k directly.
  - gamma = exp(min(sm + a, 0)/0.3) = min(exp(sm/0.3) * k/s_19', 1).
  - Errors injected at iteration t decay by ~0.55 per subsequent iteration, so
    the t=0..7 varying-theta phase runs on a 1/8 column subsample (chunks of 16
    columns every 128, DMA-friendly) with a subsample-consistent k; the 12
    constant-theta iterations run full width.  Validated vs. the f32 reference
    at <2e-3 max elementwise relative error.

The only ACT function used is Exp (the tiny per-row clip updates use DVE pow),
so there is exactly one activation-table load in the whole kernel.

Sharding: pure row-parallel, 4096 rows -> 8 cores x 512 rows.
Each core processes 4 tiles of [128 rows, 8192 cols].
"""

import os
import sys

import numpy as np

try:
    import concourse.bass as bass
except ImportError:
    sys.path.insert(0, "/opt/trn_rl_repo")
    import concourse.bass as bass  # noqa: F401

import ml_dtypes

import concourse.bacc as bacc
import concourse.tile as tile
from concourse import mybir
from concourse.bass_utils import run_bass_kernel_spmd

F32 = mybir.dt.float32
BF16 = mybir.dt.bfloat16
A = mybir.AluOpType
AF = mybir.ActivationFunctionType

# Problem constants
THETA, THETA0, T_ITERS, BETA, P_FRAC = 0.3, 4.0, 20, 0.7, 0.1
BSZ, SEQ = 4096, 8192
N_CORES = 8
ROWS_PER_CORE = BSZ // N_CORES          # 512
P = 128                                  # partitions
N_TILES = ROWS_PER_CORE // P             # 4
CHUNK = 16                               # subsample: 16 cols every 128
CHUNK_EVERY = 128
N_CHUNKS = SEQ // CHUNK_EVERY            # 64
SUB = N_CHUNKS * CHUNK                   # 1024
BIG = 1.0e30

THETAS = [max(BETA**t * THETA0, THETA) for t in range(T_ITERS)]
N_SUB_ITERS = int(os.environ.get("NORM_SUB_ITERS", "12"))
N_FULL_ITERS = int(os.environ.get("NORM_FULL_ITERS", "12"))
SUB_CONTIG = os.environ.get("NORM_SUB_CONTIG", "0") == "1"
SKIP_STT = os.environ.get("NORM_SKIP_STT", "0") == "1"


def _chunk_view(ap):
    """[P, SEQ] access pattern -> [P, N_CHUNKS, CHUNK] subsample view."""
    return ap.rearrange("p (c l) -> p c l", l=CHUNK_EVERY)[:, :, 0:CHUNK]


def build_kernel(loop_n: int = 1):
    nc = bacc.Bacc("TRN2", target_bir_lowering=False, debug=False,
                   num_devices=N_CORES)
    score_d = nc.dram_tensor("score", [ROWS_PER_CORE, SEQ], F32,
                             kind="ExternalInput")
    maskbf_d = nc.dram_tensor("maskbf", [ROWS_PER_CORE, SEQ], BF16,
                              kind="ExternalInput")
    gamma_d = nc.dram_tensor("gamma", [ROWS_PER_CORE, SEQ], F32,
                             kind="ExternalOutput")

    with tile.TileContext(nc) as tc:
        import contextlib
        loop_cm = tc.For_i(0, loop_n, 1) if loop_n > 1 else \
            contextlib.nullcontext()
        with (
            loop_cm,
            tc.tile_pool(name="smp", bufs=2) as smp,
            tc.tile_pool(name="ep", bufs=2) as ep,
            tc.tile_pool(name="mpp", bufs=2) as mpp,
            tc.tile_pool(name="junkp", bufs=2) as junkp,
            tc.tile_pool(name="ssubp", bufs=2) as ssubp,
            tc.tile_pool(name="psubp", bufs=2) as psubp,
            tc.tile_pool(name="esubp", bufs=2) as esubp,
            tc.tile_pool(name="sjunkp", bufs=2) as sjunkp,
            tc.tile_pool(name="scalars", bufs=4 * N_TILES) as scalars,
        ):
            for j in range(N_TILES):
                r0 = j * P
                # ---- DMAs ------------------------------------------------
                st = ssubp.tile([P, SUB], F32, tag="ssub")
                pt = psubp.tile([P, SUB], BF16, tag="psub")
                if SUB_CONTIG:
                    nc.sync.dma_start(out=st[:],
                                      in_=score_d.ap()[r0:r0 + P, 0:SUB])
                    nc.sync.dma_start(out=pt[:],
                                      in_=maskbf_d.ap()[r0:r0 + P, 0:SUB])
                else:
                    nc.sync.dma_start(
                        out=st[:].rearrange("p (c l) -> p c l", l=CHUNK),
                        in_=_chunk_view(score_d.ap()[r0:r0 + P, :]))
                    nc.sync.dma_start(
                        out=pt[:].rearrange("p (c l) -> p c l", l=CHUNK),
                        in_=_chunk_view(maskbf_d.ap()[r0:r0 + P, :]))
                mp = mpp.tile([P, SEQ], BF16, tag="mp")
                nc.sync.dma_start(out=mp[:], in_=maskbf_d.ap()[r0:r0 + P, :])
                sm = smp.tile([P, SEQ], F32, tag="sm")
                nc.sync.dma_start(out=sm[:], in_=score_d.ap()[r0:r0 + P, :])

                # ---- subsample: E_sub and k_sub --------------------------
                nc.vector.scalar_tensor_tensor(
                    out=st[:], in0=pt[:], scalar=0.0, in1=st[:],
                    op0=A.add, op1=A.add)
                sj = sjunkp.tile([P, SUB], BF16, tag="sjunk")
                cnt_s = scalars.tile([P, 1], F32, tag="cnts")
                nc.vector.tensor_scalar(out=sj[:], in0=pt[:],
                                        scalar1=0.0, scalar2=None,
                                        op0=A.is_equal, op1=A.add,
                                        accum_out=cnt_s[:])
                ks_t = scalars.tile([P, 1], F32, tag="ks")
                nc.vector.tensor_scalar_mul(out=ks_t[:], in0=cnt_s[:],
                                            scalar1=P_FRAC)
                rks_t = scalars.tile([P, 1], F32, tag="rks")
                nc.vector.reciprocal(out=rks_t[:], in_=ks_t[:])
                es_t = esubp.tile([P, SUB], BF16, tag="esub")
                nc.scalar.activation(out=es_t[:], in_=st[:], func=AF.Exp,
                                     scale=1.0 / THETA)

                # ---- full-width setup ------------------------------------
                if not SKIP_STT:
                    nc.vector.scalar_tensor_tensor(
                        out=sm[:], in0=mp[:], scalar=0.0, in1=sm[:],
                        op0=A.add, op1=A.add)
                junk = junkp.tile([P, SEQ], BF16, tag="junk")
                cnt = scalars.tile([P, 1], F32, tag="cnt")
                nc.vector.tensor_scalar(out=junk[:], in0=mp[:], scalar1=0.0,
                                        scalar2=None, op0=A.is_equal,
                                        op1=A.add, accum_out=cnt[:])
                k_t = scalars.tile([P, 1], F32, tag="k")
                nc.vector.tensor_scalar_mul(out=k_t[:], in0=cnt[:],
                                            scalar1=P_FRAC)
                rk = scalars.tile([P, 1], F32, tag="rk")
                nc.vector.reciprocal(out=rk[:], in_=k_t[:])
                # E = exp(sm/0.3) bf16;  G = exp(sm/0.3) f32 in place over sm
                e_t = ep.tile([P, SEQ], BF16, tag="E")
                nc.scalar.activation(out=e_t[:], in_=sm[:], func=AF.Exp,
                                     scale=1.0 / THETA)
                nc.scalar.activation(out=sm[:], in_=sm[:], func=AF.Exp,
                                     scale=1.0 / THETA)

                # ---- converge c on the subsample (from far above) --------
                c_t = None
                for t in range(N_SUB_ITERS):
                    sj = sjunkp.tile([P, SUB], BF16, tag="sjunk")
                    s_t = scalars.tile([P, 1], F32, tag="s")
                    if c_t is None:
                        nc.vector.tensor_scalar(out=sj[:], in0=es_t[:],
                                                scalar1=BIG, scalar2=None,
                                                op0=A.min, op1=A.add,
                                                accum_out=s_t[:])
                    else:
                        nc.vector.tensor_scalar(out=sj[:], in0=es_t[:],
                                                scalar1=c_t[:], scalar2=None,
                                                op0=A.min, op1=A.add,
                                                accum_out=s_t[:])
                    c_t = scalars.tile([P, 1], F32, tag="c")
                    nc.vector.tensor_scalar(out=c_t[:], in0=s_t[:],
                                            scalar1=1e-20, scalar2=rks_t[:],
                                            op0=A.add, op1=A.mult)

                # ---- polish on the full row ------------------------------
                s_t = None
                for t in range(N_FULL_ITERS):
                    cj = junkp.tile([P, SEQ], BF16, tag="junk")
                    s_t = scalars.tile([P, 1], F32, tag="s")
                    nc.vector.tensor_scalar(out=cj[:], in0=e_t[:],
                                            scalar1=c_t[:], scalar2=None,
                                            op0=A.min, op1=A.add,
                                            accum_out=s_t[:])
                    if t < N_FULL_ITERS - 1:
                        c_t = scalars.tile([P, 1], F32, tag="c")
                        nc.vector.tensor_scalar(out=c_t[:], in0=s_t[:],
                                                scalar1=1e-20, scalar2=rk[:],
                                                op0=A.add, op1=A.mult)

                # ---- gamma = min(G * k/s', 1) ----------------------------
                sp = scalars.tile([P, 1], F32, tag="sp")
                nc.vector.tensor_scalar_add(out=sp[:], in0=s_t[:],
                                            scalar1=1e-20)
                rs = scalars.tile([P, 1], F32, tag="rs")
                nc.vector.reciprocal(out=rs[:], in_=sp[:])
                ca = scalars.tile([P, 1], F32, tag="ca")
                nc.vector.tensor_scalar(out=ca[:], in0=rs[:],
                                        scalar1=k_t[:], scalar2=None,
                                        op0=A.mult, op1=A.bypass)
                nc.vector.tensor_scalar(out=sm[:], in0=sm[:], scalar1=ca[:],
                                        scalar2=1.0, op0=A.mult, op1=A.min)
                nc.sync.dma_start(out=gamma_d.ap()[r0:r0 + P, :], in_=sm[:])

    nc.compile()
    return nc


_NC_CACHE = None


def encode_mask(mask: np.ndarray) -> np.ndarray:
    """{0,1} int mask -> additive penalty {-BIG, 0} in bf16."""
    return np.where(np.asarray(mask) == 0, np.float32(-BIG),
                    np.float32(0.0)).astype(ml_dtypes.bfloat16)


def kernel(score: np.ndarray, mask: np.ndarray) -> np.ndarray:
    global _NC_CACHE
    if _NC_CACHE is None:
        _NC_CACHE = build_kernel()
    nc = _NC_CACHE

    maskpen = encode_mask(mask)
    score = np.ascontiguousarray(np.asarray(score, dtype=np.float32))

    in_maps = []
    for i in range(N_CORES):
        sl = slice(i * ROWS_PER_CORE, (i + 1) * ROWS_PER_CORE)
        in_maps.append({
            "score": score[sl],
            "maskbf": np.ascontiguousarray(maskpen[sl]),
        })
    res = run_bass_kernel_spmd(nc, in_maps, core_ids=list(range(N_CORES)))
    out = np.concatenate([res.results[i]["gamma"] for i in range(N_CORES)],
                         axis=0)
    return out.astype(np.float32)



# revision 7
# speedup vs baseline: 3.7982x; 3.7982x over previous
"""Trainium2 Bass kernel for nn_Normalizer (annealed top-k masking normalizer).

Math (see reference): the T=20 annealed-theta loop converges; the output
depends only on the fixed point c* of  s(c) = k*c  where, in exp space,
E = exp(masked_score/theta),  s(c) = sum_j min(E_j, c),  k = 0.1 * n_finite.
The scheduled trajectory's c_19 differs from c* by ~1e-4 relative, far
below the accuracy gate, so the kernel solves the fixed point directly:

  1. host: sm = fp16(score, masked -> -60000)           [halves input DMA]
  2. ACT:  E = bf16(exp(sm/theta)) per 128-row tile
  3. k = 0.1 * count(finite): DVE slice counts sm > -1000 (runs during the
     exp phase, straight off the fp16 input); ACT slice counts via
     sum(relu(1 - HUGE*E)) = width - count  (exact: masked E is exactly 0)
  4. sub phase: ~6 fixed-point iters on a 1/8 column subsample
     (16 cols every 128) read directly from E via a strided view;
     tiles {0,1} iterate on DVE (min+accum), tiles {2,3} on ACT
     (iter0: Identity+accum = plain sum; then relu-trick), so both engines
     run two independent convergence chains in parallel.
  5. full phase "FSS": three full-width s(c) passes, column-sliced across
     DVE (min+accum) and ACT (relu-trick: sum min = W*c - sum relu(c-E));
     update 1 = plain fixed point c=s/k, updates 2,3 = secant (slope from
     the last two (c, s) pairs) -- no full count passes needed.
  6. gamma = bf16(min(E * (1/c), 1)) in place over E (DVE 4x mode), DMA out
     as bf16; host upcasts to f32.

All row-scalars for the 4 tiles are batched as [128,4] (or per-group
[128,2]) f32 tiles so each scalar update is one instruction per core.
The Pool engine only supports tensor_tensor add/mult in this toolchain,
so it takes a few of the secant tensor-tensor ops and nothing else.

Sharding: pure row-parallel, 4096 rows -> 8 cores x 512 rows.
"""

import os
import sys

import numpy as np

try:
    import concourse.bass as bass  # noqa: F401
except ImportError:
    sys.path.insert(0, "/opt/trn_rl_repo")
    import concourse.bass as bass  # noqa: F401

import ml_dtypes  # noqa: F401

import concourse.bacc as bacc
import concourse.tile as tile
from concourse import mybir
from concourse.bass_utils import run_bass_kernel_spmd

F32 = mybir.dt.float32
BF16 = mybir.dt.bfloat16
FP16 = mybir.dt.float16
A = mybir.AluOpType
AF = mybir.ActivationFunctionType

THETA, P_FRAC = 0.3, 0.1
BSZ, SEQ = 4096, 8192
N_CORES = 8
ROWS_PER_CORE = BSZ // N_CORES          # 512
P = 128
N_TILES = ROWS_PER_CORE // P            # 4
CHUNK, CHUNK_EVERY = 16, 128            # subsample: 16 cols every 128
SUB = SEQ // CHUNK_EVERY * CHUNK        # 1024
BIG = 1.0e30
HH = 1.0e25                             # relu count scaling
MASKVAL = -60000.0                      # fp16-representable, exp -> 0
SM_THRESH = -1000.0                     # finite iff sm > this

N_SUB = int(os.environ.get("NORM_SUB_ITERS", "6"))
FULL_SEQ = os.environ.get("NORM_FULL_SEQ", "FSS")  # F=fixed point, S=secant
# column-slice widths (DVE vs ACT) for the full s-passes and k-passes
S_DVE = int(os.environ.get("NORM_S_DVE", "3584"))
S_ACT = SEQ - S_DVE
K_DVE = int(os.environ.get("NORM_K_DVE", "4096"))
K_ACT = SEQ - K_DVE
DEBUG = os.environ.get("NORM_DEBUG", "0") == "1"


def _sub_view(ap):
    """[P, SEQ] AP -> [P, 64, 16] strided subsample view."""
    return ap.rearrange("p (c l) -> p c l", l=CHUNK_EVERY)[:, :, 0:CHUNK]


def _sub_out(ap):
    """[P, SUB] contiguous AP -> [P, 64, 16] view (match sub input shape)."""
    return ap.rearrange("p (c l) -> p c l", l=CHUNK)


def build_kernel():
    nc = bacc.Bacc("TRN2", target_bir_lowering=False, debug=False,
                   num_devices=N_CORES)
    sm_d = nc.dram_tensor("sm", [ROWS_PER_CORE, SEQ], FP16,
                          kind="ExternalInput")
    gamma_d = nc.dram_tensor("gamma", [ROWS_PER_CORE, SEQ], BF16,
                             kind="ExternalOutput")
    dbg_d = nc.dram_tensor("dbg", [P, 64], F32,
                           kind="ExternalOutput") if DEBUG else None
    dbg_tiles = []

    def dbg(name, t, w=N_TILES):
        if DEBUG:
            dbg_tiles.append((name, t, w))

    v = nc.vector
    g = nc.gpsimd
    s = nc.scalar

    with tile.TileContext(nc) as tc:
        with (
            tc.tile_pool(name="smp", bufs=1) as smp,
            tc.tile_pool(name="ep", bufs=1) as ep,
            tc.tile_pool(name="jdp", bufs=1) as jdp,
            tc.tile_pool(name="jap", bufs=1) as jap,
            tc.tile_pool(name="jsp", bufs=1) as jsp,
            tc.tile_pool(name="scal", bufs=8) as scal,
        ):
            jD = jdp.tile([P, max(K_DVE, S_DVE, SUB)], F32, tag="jD")
            jA = jap.tile([P, max(K_ACT, S_ACT)], F32, tag="jA")
            jSD = jsp.tile([P, SUB], F32, tag="jSD")
            jSA = jsp.tile([P, SUB], F32, tag="jSA")

            cnts4 = scal.tile([P, N_TILES], F32, tag="cnts")
            kD4 = scal.tile([P, N_TILES], F32, tag="kD")
            rkA4 = scal.tile([P, N_TILES], F32, tag="rkA")

            # ---- phase A: DMA in, exp, count passes ---------------------
            E = []
            for j in range(N_TILES):
                r0 = j * P
                sm = smp.tile([P, SEQ], FP16, tag=f"sm{j % 2}")
                nc.sync.dma_start(out=sm[:], in_=sm_d.ap()[r0:r0 + P, :])
                e_t = ep.tile([P, SEQ], BF16, tag=f"E{j}")
                E.append(e_t)
                s.activation(out=e_t[:], in_=sm[:], func=AF.Exp,
                             scale=1.0 / THETA)
                # subsample count off sm (DVE; runs while ACT does exp)
                v.tensor_scalar(out=_sub_out(jSD[:]), in0=_sub_view(sm[:]),
                                scalar1=SM_THRESH, scalar2=None,
                                op0=A.is_gt, op1=A.add,
                                accum_out=cnts4[:, j:j + 1])
                # full-count DVE slice off sm
                v.tensor_scalar(out=jD[:][:, 0:K_DVE],
                                in0=sm[:][:, 0:K_DVE],
                                scalar1=SM_THRESH, scalar2=None,
                                op0=A.is_gt, op1=A.add,
                                accum_out=kD4[:, j:j + 1])
                # full-count ACT slice off E: sum(relu(1-HH*E)) = W - count
                s.activation(out=jA[:][:, 0:K_ACT],
                             in_=e_t[:][:, K_DVE:SEQ],
                             func=AF.Relu, scale=-HH, bias=1.0,
                             accum_out=rkA4[:, j:j + 1])

            # ---- k prep (batched) ---------------------------------------
            t1 = scal.tile([P, N_TILES], F32, tag="t1")
            v.scalar_tensor_tensor(out=t1[:], in0=rkA4[:], scalar=-1.0,
                                   in1=kD4[:], op0=A.mult, op1=A.add)
            cnt4 = scal.tile([P, N_TILES], F32, tag="cnt4")
            v.tensor_scalar_add(cnt4[:], t1[:], float(K_ACT))
            dbg("cnts4", cnts4)
            dbg("kD4", kD4)
            dbg("rkA4", rkA4)
            dbg("cnt4", cnt4)
            k4 = scal.tile([P, N_TILES], F32, tag="k4")
            v.tensor_scalar_mul(k4[:], cnt4[:], P_FRAC)
            rk4 = scal.tile([P, N_TILES], F32, tag="rk4")
            v.reciprocal(rk4[:], k4[:])
            k02 = scal.tile([P, N_TILES], F32, tag="k02")
            v.tensor_scalar_mul(k02[:], k4[:], 0.02)

            # per-group subsample rks = 10 / cnt_sub
            rks = []
            for grp in range(2):
                rc_ = scal.tile([P, 2], F32, tag=f"rcs{grp}")
                v.reciprocal(rc_[:], cnts4[:, 2 * grp:2 * grp + 2])
                rk_ = scal.tile([P, 2], F32, tag=f"rks{grp}")
                v.tensor_scalar_mul(rk_[:], rc_[:], 1.0 / P_FRAC)
                rks.append(rk_)

            # ---- phase B: subsample fixed point, 2 chains ---------------
            # group A = tiles {0,1} on DVE, group B = tiles {2,3} on ACT
            cA = cB = None
            for it in range(N_SUB):
                sA = scal.tile([P, 2], F32, tag="sgA")
                rB = scal.tile([P, 2], F32, tag="sgB")
                for jj in range(2):
                    v.tensor_scalar(out=_sub_out(jSD[:]),
                                    in0=_sub_view(E[jj][:]),
                                    scalar1=(BIG if it == 0
                                             else cA[:, jj:jj + 1]),
                                    scalar2=None,
                                    op0=A.min, op1=A.add,
                                    accum_out=sA[:, jj:jj + 1])
                for jj in range(2):
                    if it == 0:
                        s.activation(out=_sub_out(jSA[:]),
                                     in_=_sub_view(E[2 + jj][:]),
                                     func=AF.Identity,
                                     accum_out=rB[:, jj:jj + 1])
                    else:
                        s.activation(out=_sub_out(jSA[:]),
                                     in_=_sub_view(E[2 + jj][:]),
                                     func=AF.Relu, scale=-1.0,
                                     bias=cB[:, jj:jj + 1],
                                     accum_out=rB[:, jj:jj + 1])
                cAn = scal.tile([P, 2], F32, tag="cgA")
                v.tensor_mul(cAn[:], sA[:], rks[0][:])
                cBn = scal.tile([P, 2], F32, tag="cgB")
                if it == 0:
                    v.tensor_mul(cBn[:], rB[:], rks[1][:])
                else:
                    # s = SUB*c - r ; c' = s * rks
                    tB = scal.tile([P, 2], F32, tag="tgB")
                    v.scalar_tensor_tensor(out=tB[:], in0=cB[:],
                                           scalar=float(SUB), in1=rB[:],
                                           op0=A.mult, op1=A.subtract)
                    v.tensor_mul(cBn[:], tB[:], rks[1][:])
                cA, cB = cAn, cBn

            # merge group c into batched [P,4]
            c4 = scal.tile([P, N_TILES], F32, tag="c4m")
            v.tensor_copy(c4[:, 0:2], cA[:])
            v.tensor_copy(c4[:, 2:4], cB[:])
            dbg("c_sub", c4)

            # ---- phase C: full-width passes (FSS) -----------------------
            def full_s_pass(c_t, tag):
                sD = scal.tile([P, N_TILES], F32, tag="sD" + tag)
                rA = scal.tile([P, N_TILES], F32, tag="rA" + tag)
                for j in range(N_TILES):
                    cj = c_t[:, j:j + 1]
                    v.tensor_scalar(out=jD[:][:, 0:S_DVE],
                                    in0=E[j][:][:, 0:S_DVE],
                                    scalar1=cj, scalar2=None,
                                    op0=A.min, op1=A.add,
                                    accum_out=sD[:, j:j + 1])
                    s.activation(out=jA[:][:, 0:S_ACT],
                                 in_=E[j][:][:, S_DVE:SEQ],
                                 func=AF.Relu, scale=-1.0, bias=cj,
                                 accum_out=rA[:, j:j + 1])
                # s = sD + S_ACT*c - rA
                u1 = scal.tile([P, N_TILES], F32, tag="u1" + tag)
                v.scalar_tensor_tensor(out=u1[:], in0=c_t[:],
                                       scalar=float(S_ACT), in1=rA[:],
                                       op0=A.mult, op1=A.subtract)
                s4 = scal.tile([P, N_TILES], F32, tag="s4" + tag)
                v.tensor_add(s4[:], sD[:], u1[:])
                return s4

            cp, sp_ = None, None
            for i, stepc in enumerate(FULL_SEQ):
                s4 = full_s_pass(c4, f"f{i}")
                cn = scal.tile([P, N_TILES], F32, tag=f"c4_{i}")
                if stepc == "F":
                    v.tensor_mul(cn[:], s4[:], rk4[:])
                else:  # secant
                    dc = scal.tile([P, N_TILES], F32, tag=f"dc{i}")
                    g.tensor_sub(dc[:], c4[:], cp[:])
                    ds = scal.tile([P, N_TILES], F32, tag=f"ds{i}")
                    g.tensor_sub(ds[:], s4[:], sp_[:])
                    kc = scal.tile([P, N_TILES], F32, tag=f"kc{i}")
                    g.tensor_mul(kc[:], k4[:], c4[:])
                    dc2 = scal.tile([P, N_TILES], F32, tag=f"dc2{i}")
                    v.scalar_tensor_tensor(out=dc2[:], in0=c4[:],
                                           scalar=1e-30, in1=dc[:],
                                           op0=A.mult, op1=A.add)
                    rdc = scal.tile([P, N_TILES], F32, tag=f"rdc{i}")
                    v.reciprocal(rdc[:], dc2[:])
                    m_ = scal.tile([P, N_TILES], F32, tag=f"m{i}")
                    v.tensor_mul(m_[:], ds[:], rdc[:])
                    den = scal.tile([P, N_TILES], F32, tag=f"den{i}")
                    v.tensor_sub(den[:], k4[:], m_[:])
                    den2 = scal.tile([P, N_TILES], F32, tag=f"den2{i}")
                    v.tensor_max(den2[:], den[:], k02[:])
                    rden = scal.tile([P, N_TILES], F32, tag=f"rden{i}")
                    v.reciprocal(rden[:], den2[:])
                    num = scal.tile([P, N_TILES], F32, tag=f"num{i}")
                    v.tensor_sub(num[:], s4[:], kc[:])
                    tq = scal.tile([P, N_TILES], F32, tag=f"tq{i}")
                    v.tensor_mul(tq[:], num[:], rden[:])
                    v.tensor_add(cn[:], c4[:], tq[:])
                dbg(f"s4_{i}", s4)
                dbg(f"c4_{i}", cn)
                cp, sp_ = c4, s4
                c4 = cn

            # ---- phase D: gamma (in place over E), DMA out --------------
            rc4 = scal.tile([P, N_TILES], F32, tag="rc4")
            v.reciprocal(rc4[:], c4[:])
            if DEBUG:
                off = 0
                for name, t, w in dbg_tiles:
                    nc.sync.dma_start(out=dbg_d.ap()[:, off:off + w],
                                      in_=t[:])
                    off += w
            for j in range(N_TILES):
                r0 = j * P
                v.tensor_scalar(out=E[j][:], in0=E[j][:],
                                scalar1=rc4[:, j:j + 1], scalar2=1.0,
                                op0=A.mult, op1=A.min)
                nc.sync.dma_start(out=gamma_d.ap()[r0:r0 + P, :],
                                  in_=E[j][:])

    nc.compile()
    return nc


_NC_CACHE = None


def prep_sm(score: np.ndarray, mask: np.ndarray) -> np.ndarray:
    """host-side dtype prep: masked score in fp16 (elementwise only)."""
    return np.where(np.asarray(mask) == 0, np.float16(MASKVAL),
                    np.asarray(score).astype(np.float16))


def kernel(score: np.ndarray, mask: np.ndarray) -> np.ndarray:
    global _NC_CACHE
    if _NC_CACHE is None:
        _NC_CACHE = build_kernel()
    nc = _NC_CACHE

    sm16 = np.ascontiguousarray(prep_sm(score, mask))
    in_maps = []
    for i in range(N_CORES):
        sl = slice(i * ROWS_PER_CORE, (i + 1) * ROWS_PER_CORE)
        in_maps.append({"sm": sm16[sl]})
    res = run_bass_kernel_spmd(nc, in_maps, core_ids=list(range(N_CORES)))
    out = np.concatenate([res.results[i]["gamma"] for i in range(N_CORES)],
                         axis=0)
    return out.astype(np.float32)


# revision 8
# speedup vs baseline: 4.3977x; 1.1579x over previous
"""Trainium2 Bass kernel for nn_Normalizer (annealed top-k masking normalizer).

Math (see reference): the T=20 annealed-theta loop converges; the output
depends only on the fixed point c* of  s(c) = k*c  where, in exp space,
E = exp(masked_score/theta),  s(c) = sum_j min(E_j, c),  k = 0.1 * n_finite.
The scheduled trajectory's c_19 differs from c* by ~1e-4 relative, far
below the accuracy gate, so the kernel solves the fixed point directly:

  1. host: sm = fp16(score, masked -> -60000)           [halves input DMA]
  2. ACT:  E = bf16(exp(sm/theta)) per 128-row tile, in column halves so
     compute starts as soon as the first half-DMA lands
  3. k = 0.1 * count(finite): DVE slice counts sm > -1000 (runs during the
     exp phase, straight off the fp16 input); ACT slice counts via
     sum(relu(1 - HUGE*E)) = width - count (exact: masked E is exactly 0);
     the ACT count slices are emitted late so they fill ACT idle gaps --
     they are only needed by the first full-width update.
  4. sub phase: 5 fixed-point iters on a 1/16 column subsample
     (8 cols every 128) read directly from E via a strided view;
     tiles {0,1} iterate on DVE (min+accum), tiles {2,3} on ACT
     (iter0: Identity+accum = plain sum; then relu-trick) with their
     scalar updates on the otherwise-idle Pool engine.
  5. full phase "FSS": three full-width s(c) passes, column-sliced across
     DVE (min+accum) and ACT (relu-trick: sum min = W*c - sum relu(c-E));
     update 1 = plain fixed point c=s/k, updates 2,3 = secant (slope from
     the last two (c, s) pairs) -- no full count passes needed.  The
     c-only secant inputs (dc, dc2, rdc, kc) are computed during the
     s-pass on Pool/DVE so the post-pass critical chain is short.
  6. gamma = bf16(min(E * (1/c), 1)) in place over E (DVE 4x mode, in
     halves), DMA out as bf16; host upcasts to f32.

All row-scalars for the 4 tiles are batched as [128,4] (or per-group
[128,2]) f32 tiles so each scalar update is one instruction per core.
The Pool engine only supports tensor_tensor add/mult/sub + memset in this
toolchain, so it gets exactly those.

Sharding: pure row-parallel, 4096 rows -> 8 cores x 512 rows.
"""

import os
import sys

import numpy as np

try:
    import concourse.bass as bass  # noqa: F401
except ImportError:
    sys.path.insert(0, "/opt/trn_rl_repo")
    import concourse.bass as bass  # noqa: F401

import ml_dtypes  # noqa: F401

import concourse.bacc as bacc
import concourse.tile as tile
from concourse import mybir
from concourse.bass_utils import run_bass_kernel_spmd

F32 = mybir.dt.float32
BF16 = mybir.dt.bfloat16
FP16 = mybir.dt.float16
A = mybir.AluOpType
AF = mybir.ActivationFunctionType

THETA, P_FRAC = 0.3, 0.1
BSZ, SEQ = 4096, 8192
N_CORES = 8
ROWS_PER_CORE = BSZ // N_CORES          # 512
P = 128
N_TILES = ROWS_PER_CORE // P            # 4
HALF = SEQ // 2
CHUNK, CHUNK_EVERY = 8, 128             # subsample: 8 cols every 128
SUB = SEQ // CHUNK_EVERY * CHUNK        # 512
BIG = 1.0e30
HH = 1.0e25                             # relu count scaling
MASKVAL = -60000.0                      # fp16-representable, exp -> 0
SM_THRESH = -1000.0                     # finite iff sm > this

N_SUB = int(os.environ.get("NORM_SUB_ITERS", "5"))
FULL_SEQ = os.environ.get("NORM_FULL_SEQ", "FSS")  # F=fixed point, S=secant
# column-slice widths (DVE vs ACT) for the full s-passes and k-passes
S_DVE = int(os.environ.get("NORM_S_DVE", "3712"))
S_ACT = SEQ - S_DVE
K_DVE = int(os.environ.get("NORM_K_DVE", "4096"))
K_ACT = SEQ - K_DVE
DEBUG = os.environ.get("NORM_DEBUG", "0") == "1"


def _sub_view(ap):
    """[P, SEQ] AP -> [P, 64, CHUNK] strided subsample view."""
    return ap.rearrange("p (c l) -> p c l", l=CHUNK_EVERY)[:, :, 0:CHUNK]


def _sub_out(ap):
    """[P, SUB] contiguous AP -> [P, 64, CHUNK] view."""
    return ap.rearrange("p (c l) -> p c l", l=CHUNK)


def build_kernel():
    nc = bacc.Bacc("TRN2", target_bir_lowering=False, debug=False,
                   num_devices=N_CORES)
    sm_d = nc.dram_tensor("sm", [ROWS_PER_CORE, SEQ], FP16,
                          kind="ExternalInput")
    gamma_d = nc.dram_tensor("gamma", [ROWS_PER_CORE, SEQ], BF16,
                             kind="ExternalOutput")
    dbg_d = nc.dram_tensor("dbg", [P, 64], F32,
                           kind="ExternalOutput") if DEBUG else None
    dbg_tiles = []

    def dbg(name, t, w=N_TILES):
        if DEBUG:
            dbg_tiles.append((name, t, w))

    v = nc.vector
    g = nc.gpsimd
    s = nc.scalar

    with tile.TileContext(nc) as tc:
        with (
            tc.tile_pool(name="smp", bufs=1) as smp,
            tc.tile_pool(name="ep", bufs=1) as ep,
            tc.tile_pool(name="jdp", bufs=1) as jdp,
            tc.tile_pool(name="jap", bufs=1) as jap,
            tc.tile_pool(name="jsp", bufs=1) as jsp,
            tc.tile_pool(name="scal", bufs=8) as scal,
        ):
            jD = jdp.tile([P, max(K_DVE, S_DVE, SUB)], F32, tag="jD")
            jA = jap.tile([P, max(K_ACT, S_ACT)], F32, tag="jA")
            jSD = jsp.tile([P, SUB], F32, tag="jSD")
            jSA = jsp.tile([P, SUB], F32, tag="jSA")

            cnts4 = scal.tile([P, N_TILES], F32, tag="cnts")
            kD4 = scal.tile([P, N_TILES], F32, tag="kD")
            rkA4 = scal.tile([P, N_TILES], F32, tag="rkA")
            # pool-side constants
            cSUB = scal.tile([P, 2], F32, tag="cSUB")
            g.memset(cSUB[:], float(SUB))
            eps30 = scal.tile([P, N_TILES], F32, tag="eps30")
            g.memset(eps30[:], 1e-30)

            # ---- phase A: DMA in (halves), exp (halves), DVE counts -----
            E = []
            sms = []
            for j in range(N_TILES):
                r0 = j * P
                sm = smp.tile([P, SEQ], FP16, tag=f"sm{j % 2}")
                sms.append(sm)
                nc.sync.dma_start(out=sm[:][:, 0:HALF],
                                  in_=sm_d.ap()[r0:r0 + P, 0:HALF])
                nc.sync.dma_start(out=sm[:][:, HALF:SEQ],
                                  in_=sm_d.ap()[r0:r0 + P, HALF:SEQ])
                e_t = ep.tile([P, SEQ], BF16, tag=f"E{j}")
                E.append(e_t)
                s.activation(out=e_t[:][:, 0:HALF], in_=sm[:][:, 0:HALF],
                             func=AF.Exp, scale=1.0 / THETA)
                s.activation(out=e_t[:][:, HALF:SEQ], in_=sm[:][:, HALF:SEQ],
                             func=AF.Exp, scale=1.0 / THETA)
                # full-count DVE slice off sm (first half only -> early)
                v.tensor_scalar(out=jD[:][:, 0:K_DVE],
                                in0=sm[:][:, 0:K_DVE],
                                scalar1=SM_THRESH, scalar2=None,
                                op0=A.is_gt, op1=A.add,
                                accum_out=kD4[:, j:j + 1])
                # subsample count off sm
                v.tensor_scalar(out=_sub_out(jSD[:]), in0=_sub_view(sm[:]),
                                scalar1=SM_THRESH, scalar2=None,
                                op0=A.is_gt, op1=A.add,
                                accum_out=cnts4[:, j:j + 1])

            # per-group subsample rks = 10 / cnt_sub
            rks = []
            for grp in range(2):
                rc_ = scal.tile([P, 2], F32, tag=f"rcs{grp}")
                v.reciprocal(rc_[:], cnts4[:, 2 * grp:2 * grp + 2])
                rk_ = scal.tile([P, 2], F32, tag=f"rks{grp}")
                v.tensor_scalar_mul(rk_[:], rc_[:], 1.0 / P_FRAC)
                rks.append(rk_)

            # ---- phase B: subsample fixed point, 2 chains ---------------
            # group A = tiles {0,1} on DVE, group B = tiles {2,3} on ACT
            # (B's scalar updates on Pool)
            cA = cB = None
            for it in range(N_SUB):
                sA = scal.tile([P, 2], F32, tag="sgA")
                rB = scal.tile([P, 2], F32, tag="sgB")
                for jj in range(2):
                    v.tensor_scalar(out=_sub_out(jSD[:]),
                                    in0=_sub_view(E[jj][:]),
                                    scalar1=(BIG if it == 0
                                             else cA[:, jj:jj + 1]),
                                    scalar2=None,
                                    op0=A.min, op1=A.add,
                                    accum_out=sA[:, jj:jj + 1])
                for jj in range(2):
                    if it == 0:
                        s.activation(out=_sub_out(jSA[:]),
                                     in_=_sub_view(E[2 + jj][:]),
                                     func=AF.Identity,
                                     accum_out=rB[:, jj:jj + 1])
                    else:
                        s.activation(out=_sub_out(jSA[:]),
                                     in_=_sub_view(E[2 + jj][:]),
                                     func=AF.Relu, scale=-1.0,
                                     bias=cB[:, jj:jj + 1],
                                     accum_out=rB[:, jj:jj + 1])
                cAn = scal.tile([P, 2], F32, tag="cgA")
                v.tensor_mul(cAn[:], sA[:], rks[0][:])
                cBn = scal.tile([P, 2], F32, tag="cgB")
                if it == 0:
                    g.tensor_mul(cBn[:], rB[:], rks[1][:])
                else:
                    # s = SUB*c - r ; c' = s * rks   (all on Pool)
                    uB = scal.tile([P, 2], F32, tag="ugB")
                    g.tensor_mul(uB[:], cB[:], cSUB[:])
                    tB = scal.tile([P, 2], F32, tag="tgB")
                    g.tensor_sub(tB[:], uB[:], rB[:])
                    g.tensor_mul(cBn[:], tB[:], rks[1][:])
                cA, cB = cAn, cBn

            # merge group c into batched [P,4]
            c4 = scal.tile([P, N_TILES], F32, tag="c4m")
            v.tensor_copy(c4[:, 0:2], cA[:])
            v.tensor_copy(c4[:, 2:4], cB[:])
            dbg("c_sub", c4)

            # ---- ACT count slices (fill ACT gaps; needed only by the
            # first full update) + k prep --------------------------------
            for j in range(N_TILES):
                s.activation(out=jA[:][:, 0:K_ACT],
                             in_=E[j][:][:, K_DVE:SEQ],
                             func=AF.Relu, scale=-HH, bias=1.0,
                             accum_out=rkA4[:, j:j + 1])
            t1 = scal.tile([P, N_TILES], F32, tag="t1")
            v.scalar_tensor_tensor(out=t1[:], in0=rkA4[:], scalar=-1.0,
                                   in1=kD4[:], op0=A.mult, op1=A.add)
            cnt4 = scal.tile([P, N_TILES], F32, tag="cnt4")
            v.tensor_scalar_add(cnt4[:], t1[:], float(K_ACT))
            dbg("cnts4", cnts4)
            dbg("kD4", kD4)
            dbg("rkA4", rkA4)
            dbg("cnt4", cnt4)
            k4 = scal.tile([P, N_TILES], F32, tag="k4")
            v.tensor_scalar_mul(k4[:], cnt4[:], P_FRAC)
            rk4 = scal.tile([P, N_TILES], F32, tag="rk4")
            v.reciprocal(rk4[:], k4[:])
            k02 = scal.tile([P, N_TILES], F32, tag="k02")
            v.tensor_scalar_mul(k02[:], k4[:], 0.02)

            # ---- phase C: full-width passes (FSS) -----------------------
            def full_s_pass(c_t, tag):
                sD = scal.tile([P, N_TILES], F32, tag="sD" + tag)
                rA = scal.tile([P, N_TILES], F32, tag="rA" + tag)
                for j in range(N_TILES):
                    cj = c_t[:, j:j + 1]
                    v.tensor_scalar(out=jD[:][:, 0:S_DVE],
                                    in0=E[j][:][:, 0:S_DVE],
                                    scalar1=cj, scalar2=None,
                                    op0=A.min, op1=A.add,
                                    accum_out=sD[:, j:j + 1])
                    s.activation(out=jA[:][:, 0:S_ACT],
                                 in_=E[j][:][:, S_DVE:SEQ],
                                 func=AF.Relu, scale=-1.0, bias=cj,
                                 accum_out=rA[:, j:j + 1])
                # s = sD + S_ACT*c - rA
                u1 = scal.tile([P, N_TILES], F32, tag="u1" + tag)
                v.scalar_tensor_tensor(out=u1[:], in0=c_t[:],
                                       scalar=float(S_ACT), in1=rA[:],
                                       op0=A.mult, op1=A.subtract)
                s4 = scal.tile([P, N_TILES], F32, tag="s4" + tag)
                v.tensor_add(s4[:], sD[:], u1[:])
                return s4

            cp, sp_ = None, None
            for i, stepc in enumerate(FULL_SEQ):
                if stepc != "F":
                    # c-only secant inputs: run during the s-pass
                    dc = scal.tile([P, N_TILES], F32, tag=f"dc{i}")
                    g.tensor_sub(dc[:], c4[:], cp[:])
                    ec = scal.tile([P, N_TILES], F32, tag=f"ec{i}")
                    g.tensor_mul(ec[:], c4[:], eps30[:])
                    dc2 = scal.tile([P, N_TILES], F32, tag=f"dc2{i}")
                    g.tensor_add(dc2[:], dc[:], ec[:])
                    kc = scal.tile([P, N_TILES], F32, tag=f"kc{i}")
                    g.tensor_mul(kc[:], k4[:], c4[:])
                    rdc = scal.tile([P, N_TILES], F32, tag=f"rdc{i}")
                    v.reciprocal(rdc[:], dc2[:])
                s4 = full_s_pass(c4, f"f{i}")
                cn = scal.tile([P, N_TILES], F32, tag=f"c4_{i}")
                if stepc == "F":
                    v.tensor_mul(cn[:], s4[:], rk4[:])
                else:  # secant, post-pass chain
                    ds = scal.tile([P, N_TILES], F32, tag=f"ds{i}")
                    v.tensor_sub(ds[:], s4[:], sp_[:])
                    m_ = scal.tile([P, N_TILES], F32, tag=f"m{i}")
                    v.tensor_mul(m_[:], ds[:], rdc[:])
                    den = scal.tile([P, N_TILES], F32, tag=f"den{i}")
                    v.tensor_sub(den[:], k4[:], m_[:])
                    den2 = scal.tile([P, N_TILES], F32, tag=f"den2{i}")
                    v.tensor_max(den2[:], den[:], k02[:])
                    rden = scal.tile([P, N_TILES], F32, tag=f"rden{i}")
                    v.reciprocal(rden[:], den2[:])
                    num = scal.tile([P, N_TILES], F32, tag=f"num{i}")
                    v.tensor_sub(num[:], s4[:], kc[:])
                    tq = scal.tile([P, N_TILES], F32, tag=f"tq{i}")
                    v.tensor_mul(tq[:], num[:], rden[:])
                    v.tensor_add(cn[:], c4[:], tq[:])
                dbg(f"s4_{i}", s4)
                dbg(f"c4_{i}", cn)
                cp, sp_ = c4, s4
                c4 = cn

            # ---- phase D: gamma (in place over E, halves), DMA out ------
            rc4 = scal.tile([P, N_TILES], F32, tag="rc4")
            v.reciprocal(rc4[:], c4[:])
            if DEBUG:
                off = 0
                for name, t, w in dbg_tiles:
                    nc.sync.dma_start(out=dbg_d.ap()[:, off:off + w],
                                      in_=t[:])
                    off += w
            for j in range(N_TILES):
                r0 = j * P
                for h0, h1 in ((0, HALF), (HALF, SEQ)):
                    v.tensor_scalar(out=E[j][:][:, h0:h1],
                                    in0=E[j][:][:, h0:h1],
                                    scalar1=rc4[:, j:j + 1], scalar2=1.0,
                                    op0=A.mult, op1=A.min)
                    nc.sync.dma_start(out=gamma_d.ap()[r0:r0 + P, h0:h1],
                                      in_=E[j][:][:, h0:h1])

    nc.compile()
    return nc


_NC_CACHE = None


def prep_sm(score: np.ndarray, mask: np.ndarray) -> np.ndarray:
    """host-side dtype prep: masked score in fp16 (elementwise only)."""
    return np.where(np.asarray(mask) == 0, np.float16(MASKVAL),
                    np.asarray(score).astype(np.float16))


def kernel(score: np.ndarray, mask: np.ndarray) -> np.ndarray:
    global _NC_CACHE
    if _NC_CACHE is None:
        _NC_CACHE = build_kernel()
    nc = _NC_CACHE

    sm16 = np.ascontiguousarray(prep_sm(score, mask))
    in_maps = []
    for i in range(N_CORES):
        sl = slice(i * ROWS_PER_CORE, (i + 1) * ROWS_PER_CORE)
        in_maps.append({"sm": sm16[sl]})
    res = run_bass_kernel_spmd(nc, in_maps, core_ids=list(range(N_CORES)))
    out = np.concatenate([res.results[i]["gamma"] for i in range(N_CORES)],
                         axis=0)
    return out.astype(np.float32)
